# revision 31
# baseline (speedup 1.0000x reference)
"""Multi-head attention (B=8, N=1024, C=1024, H=16) on 8 TRN2 NeuronCores.

Sharding: data-parallel over batch B=8 -> one batch element per core.
Each core computes, for its batch element:
    qkv = x @ qkv_w.T ; q,k,v split ; per-head softmax(q k^T / sqrt(hd)) v

Device-side layout (all matmuls contract over the SBUF partition dim):
  - host passes xT = x[b].T (bf16)     [C, N]   (c on partitions)
  - host passes wqk packed per head-PAIR: [pair, p, co, 256] (q cols 0:128,
    k cols 128:256) so psq/psk partitions are the pair's stacked head dims
  - qT/kT computed transposed          [128, n] (pair head-dims on partitions)
  - v computed in natural layout       [n, dv], augmented with a ones-column
    so the PV matmul also yields the softmax denominator (row 64)
  - scores S^T=[j,i]: per j-tile the TWO heads' matmuls are emitted
    back-to-back as 64-row PE tiles (tile_position (0,0)/(64,0)) so they run
    CONCURRENTLY on the two halves of the PE array (~2x score throughput)
  - exp fused into the PSUM->SBUF copy on the scalar engine (bf16 out,
    128 ACTs of [128,1024]); its ~147us stream runs just under the PE
  - the v-projection and next-pair qk-projections are chopped into
    [128,512]-output 8-matmul "quarters" and dripped into the chunk stream
    instead of running as a ~34us serial prologue (the early chunks have
    PE slack while the exp pipeline fills)
  - exp table preloaded, gpsimd partition_broadcast library prewarmed, and
    PE warmed with dummy matmuls during the input DMA window; input DMAs
    ordered by first use and spread across four engine queues (xT tiles in
    parallel) so the pair-0 projection is never DMA-paced
  - normalization per (pair, i-block), a short DMA-free chain read straight
    from the PV PSUM (no stage copies, no DRAM bounce):
      g3 slot:  DVE reciprocal_approx_fast on the [1,512] denominator row
                (custom DVE op, ~5x the plain reciprocal, no table/library)
      +1 slot:  gpsimd partition_broadcast -> [64,512]
      +2 slot:  DVE multiply (PV psum rows 0:64 x broadcast) -> DMA out
    The same chain is used for every pair including the last, so the tail
    after the final matmul is only ~4-5us of chain latency.
  - host transposes the returned outT back to [n, c]

PSUM (8 banks): shared tag "ps" 3 x [128,1024] (warmup, score chunks,
projection quarters) = 6 banks, tag "pv" 2 x [65,512] = 2 banks.
"""

import sys
from collections import deque

sys.path.insert(0, "/opt/trn_rl_repo")

import ml_dtypes
import numpy as np

import concourse.bacc as bacc
import concourse.mybir as mybir
import concourse.tile as tile
from concourse.bass_utils import run_bass_kernel_spmd

F32 = mybir.dt.float32
BF16 = mybir.dt.bfloat16
EXP = mybir.ActivationFunctionType.Exp

N = 1024  # tokens
C = 1024  # channels
H = 16    # heads
HD = 64   # head dim
CT = 8    # c tiles of 128
SCALE = HD ** -0.5
LAG = 4   # chunks the PV stream lags behind the score stream


def build_nc():
    nc = bacc.Bacc(None, target_bir_lowering=False)
    xT_ext = nc.declare_dram_parameter("xT", [C, N], BF16, isOutput=False)
    # host-packed qk weights: [pair, p, co, 256] (q cols 0:128, k cols 128:256)
    wqk_ext = nc.declare_dram_parameter("wqk", [8, 128, CT, 256], BF16,
                                        isOutput=False)
    wv_ext = nc.declare_dram_parameter("wv", [C, C], BF16, isOutput=False)
    outT_ext = nc.declare_dram_parameter("outT", [C, N], F32, isOutput=True)

    xT3 = xT_ext.rearrange("(co p) n -> p co n", p=128)    # [128, 8, 1024]
    wv3 = wv_ext.rearrange("(co p) d -> p co d", p=128)    # [128, 8, 1024]

    with tile.TileContext(nc) as tc:
        with (
            tc.tile_pool(name="singles", bufs=1) as singles,
            tc.tile_pool(name="psum", bufs=1, space="PSUM") as psum,
            tc.tile_pool(name="qkpool", bufs=2) as qkpool,
            tc.tile_pool(name="epool", bufs=9) as epool,
            tc.tile_pool(name="opool", bufs=2) as opool,
        ):
            # ---- memsets first: wsb feeds the PE warmup, scratch feeds the
            # exp-table preload and the partition_broadcast library prewarm ----
            wsb = singles.tile([128, 512], BF16)
            nc.vector.memset(wsb, 0.0)
            scratch = singles.tile([1, 16], F32)
            nc.vector.memset(scratch, 0.0)

            # ---- input DMAs: only sync (SP) and scalar (Activation) have
            # HARDWARE DMA rings (~100GB/s each); gpsimd's software path is
            # slow, so it carries just two early xT tiles.  scalar's issues
            # all land in the prologue window (a backed-up ring stalls the
            # queue, which later carries the exp stream). ----
            wqk_head = singles.tile([128, 2, 256], BF16)
            nc.sync.dma_start(out=wqk_head, in_=wqk_ext[0, :, 0:2])
            xT_sb = singles.tile([128, CT, N], BF16)
            nc.scalar.dma_start(out=xT_sb[:, 0, :], in_=xT3[:, 0, :])
            nc.gpsimd.dma_start(out=xT_sb[:, 1, :], in_=xT3[:, 1, :])
            nc.scalar.dma_start(out=xT_sb[:, 2, :], in_=xT3[:, 2, :])
            wqk_rest = singles.tile([128, 6, 256], BF16)
            nc.sync.dma_start(out=wqk_rest, in_=wqk_ext[0, :, 2:8])
            nc.gpsimd.dma_start(out=xT_sb[:, 3, :], in_=xT3[:, 3, :])
            for ct in range(4, CT):
                nc.sync.dma_start(out=xT_sb[:, ct, :], in_=xT3[:, ct, :])

            # ---- exp table preload (tiny ACT) + gpsimd partition_broadcast
            # library prewarm, both during the DMA window ----
            scratch_o = singles.tile([1, 16], BF16)
            nc.scalar.activation(out=scratch_o, in_=scratch, func=EXP,
                                 scale=1.0)
            pb_warm = singles.tile([2, 16], F32)
            nc.gpsimd.partition_broadcast(pb_warm, scratch)

            # ---- PE warmup: dummy matmuls during the input-DMA window so
            # the HAM clock gate is already ramping when real work starts ----
            wps = psum.tile([128, 1024], F32, tag="ps", bufs=3, name="warm")
            for _ in range(4):
                nc.tensor.matmul(wps[:, 0:512], wsb[:, 0:128], wsb,
                                 start=True, stop=True,
                                 skip_group_check=True)

            # ---- remaining bulk loads on sync, first-use order: wv dvb0,
            # wqk pair 1, wv dvb1 evens, wqk pairs 2-7 (wv dvb1 odds are
            # issued from the scalar queue inside the first chunk, where the
            # ACT engine is idle anyway -- see the chunk loop) ----
            wv_sb = singles.tile([128, CT, C], BF16)
            for ct in range(CT):
                nc.sync.dma_start(out=wv_sb[:, ct, 0:512],
                                  in_=wv3[:, ct, 0:512])
            wqk_tiles = {}

            def load_wqk(t, q=None):
                w = singles.tile([128, CT, 256], BF16)
                (q or nc.sync).dma_start(out=w, in_=wqk_ext[t])
                wqk_tiles[t] = w

            load_wqk(1)
            load_wqk(2, q=nc.scalar)  # scalar ring has slack; sync is ~10us
            for ct in (0, 2, 4, 6):   # late for wqk2's ~45us deadline
                nc.sync.dma_start(out=wv_sb[:, ct, 512:1024],
                                  in_=wv3[:, ct, 512:1024])
            for t in range(3, 8):
                load_wqk(t)

            def wqk0_slice(ct, off):
                if ct < 2:
                    return wqk_head[:, ct, off:off + 128]
                return wqk_rest[:, ct - 2, off:off + 128]

            # v_aug[p, nt, h, 0:64] = v head h rows; v_aug[p, nt, h, 64] = 1.0
            v_aug = singles.tile([128, CT, H, HD + 1], BF16)
            ones16 = singles.tile([128, H], F32)
            nc.vector.memset(ones16, 1.0)
            nc.vector.tensor_copy(
                v_aug[:, :, :, HD],
                ones16[:, None, :].to_broadcast([128, CT, H]),
            )

            def ps_big(name):
                return psum.tile([128, 1024], F32, tag="ps", bufs=3, name=name)

            # ---- pair-0 qk projection (q and k interleaved per c-tile so
            # both ride the incoming xT DMA wave) ----
            psq = ps_big("qk0q")
            psk = ps_big("qk0k")
            for ct in range(CT):
                for ps, off in ((psq, 0), (psk, 128)):
                    for nb in range(2):
                        nc.tensor.matmul(
                            ps[:, nb * 512:(nb + 1) * 512],
                            wqk0_slice(ct, off),
                            xT_sb[:, ct, nb * 512:(nb + 1) * 512],
                            start=(ct == 0),
                            stop=(ct == CT - 1),
                            skip_group_check=True,
                        )
            # qT0 via the (idle) ACT engine, kT0 via the DVE -- concurrent,
            # halving the serial PSUM->SBUF gap between pair-0's last matmul
            # and the first score chunk
            qT0 = qkpool.tile([128, N], BF16, tag="qT", name="qT")
            kT0 = qkpool.tile([128, N], BF16, tag="kT", name="kT")
            nc.scalar.copy(qT0, psq)
            nc.vector.tensor_copy(kT0, psk)

            # ---- projection "quarters": 8-matmul bursts with a [128,512]
            # output, dripped into the chunk stream ----
            qk_targets = {}

            def emit_quarter(spec):
                qps = psum.tile([128, 1024], F32, tag="ps", bufs=3,
                                name="qtr")
                if spec[0] == "qk":
                    _, tn, half, nb = spec
                    dst = qk_targets[tn][half]
                    off = 128 * half
                    w = wqk_tiles[tn]
                    for ct in range(CT):
                        nc.tensor.matmul(
                            qps[:, 0:512],
                            w[:, ct, off:off + 128],
                            xT_sb[:, ct, nb * 512:(nb + 1) * 512],
                            start=(ct == 0),
                            stop=(ct == CT - 1),
                            skip_group_check=True,
                        )
                    nc.vector.tensor_copy(dst[:, nb * 512:(nb + 1) * 512],
                                          qps[:, 0:512])
                else:
                    _, nt, dvb = spec
                    for ct in range(CT):
                        nc.tensor.matmul(
                            qps[:, 0:512],
                            xT_sb[:, ct, nt * 128:(nt + 1) * 128],
                            wv_sb[:, ct, dvb * 512:(dvb + 1) * 512],
                            start=(ct == 0),
                            stop=(ct == CT - 1),
                            skip_group_check=True,
                        )
                    h0 = dvb * 8
                    nc.vector.tensor_copy(
                        v_aug[:, nt, h0:h0 + 8, 0:HD],
                        qps[:, 0:512].rearrange("p (h e) -> p h e", h=8),
                    )

            # quarter schedule by global chunk index c = t*8 + ib*4 + g.
            # t0: v-dvb0 for all 8 token-tiles + pair-1 qk, each as late as
            # its consumer allows (v(nt) feeds the PV of chunk nt//2+LAG;
            # qk1 feeds pair-1 scores at chunk 8) so none outruns the input
            # DMA stream; later t's: pair-(t+1) qk at even slots, v-dvb1
            # dripped during t1-t3.
            sched = {
                0: [("v", 0, 0), ("v", 1, 0)],
                1: [("v", 2, 0), ("qk", 1, 0, 0)],
                2: [("v", 3, 0), ("v", 4, 0)],
                3: [("v", 5, 0), ("qk", 1, 0, 1)],
                4: [("v", 6, 0), ("v", 7, 0)],
                5: [("qk", 1, 1, 0)],
                6: [("qk", 1, 1, 1)],
            }
            vd1 = deque(("v", nt, 1) for nt in range(CT))
            for t in range(1, 7):
                base = t * 8
                for i, (half, nb) in enumerate(
                        ((0, 0), (0, 1), (1, 0), (1, 1))):
                    sched.setdefault(base + 2 * i, []).append(
                        ("qk", t + 1, half, nb))
                # v-dvb1 rides t2-t3 (its wv arrives ~60us; true deadline is
                # t4's PV at ~120us)
                if t in (2, 3):
                    for i in (1, 3, 5, 7):
                        if vd1:
                            sched.setdefault(base + i, []).append(
                                vd1.popleft())
            assert not vd1

            # ---- software-pipelined attention stream ----
            pending = deque()  # entries: list of emitters for one chunk

            def drain(keep):
                while len(pending) > keep:
                    for e in pending.popleft():
                        e()

            def pv_emit(es, g, pvs, t):
                def emit():
                    for hh in range(2):
                        h = 2 * t + hh
                        for jh in range(2):
                            jt = 2 * g + jh
                            nc.tensor.matmul(
                                pvs[hh],
                                v_aug[:, jt, h, :],
                                es[hh][:, jh, :],
                                start=(g == 0 and jh == 0),
                                stop=(g == 3 and jh == 1),
                                skip_group_check=True,
                            )
                return emit

            # Normalization, a short DMA-free chain straight off the PV psum
            # (row 64 is the softmax denominator), spread over 3 chunk-slots
            # so no in-order engine queue ever blocks on an upstream engine:
            #   g3 slot:  DVE reciprocal_approx_fast on the [1,512] row
            #   +1 slot:  gpsimd partition_broadcast -> [64,512]
            #   +2 slot:  DVE multiply (psum x broadcast) + per-head DMA out
            # The multiply is the last reader of the pv psum tiles, freeing
            # them 3 chunk-slots before the slot is needed again (bufs=2).
            def rcp_emit(pvs, store):
                def emit():
                    for k in range(2):
                        # custom-DVE ops misread PSUM: bounce the denominator
                        # row through SBUF with a plain copy first
                        row = opool.tile([1, 512], F32, tag=f"row{k}",
                                         bufs=2, name="row")
                        nc.vector.tensor_copy(row, pvs[k][HD:HD + 1, :])
                        rcp = opool.tile([1, 512], F32, tag=f"rcp{k}",
                                         bufs=2, name="rcp")
                        nc.vector.reciprocal_approx_fast(out=rcp, in_=row)
                        store[f"rcp{k}"] = rcp
                return emit

            def bcast_emit(store):
                def emit():
                    for k in range(2):
                        bc = opool.tile([HD, 512], F32, tag=f"bc{k}",
                                        bufs=2, name="bc")
                        nc.gpsimd.partition_broadcast(bc, store[f"rcp{k}"])
                        store[f"bc{k}"] = bc
                return emit

            def mul_out_emit(pvs, store, t, ibs):
                """The multiply stays on the DVE (gpsimd has no PSUM port),
                but is scheduled one extra chunk-slot after the broadcast so
                its wait on gpsimd is already satisfied when the DVE reaches
                it -- otherwise the wait head-of-line-blocks the PE-critical
                quarter copies (observed as a ~3.5us PE gap + HAM re-throttle
                at every other ib boundary)."""
                def emit():
                    for k in range(2):
                        # bufs=4: early stores queue behind the input tail
                        # on the sync ring; 4 osb slots give the mul ~37us
                        # of slack before it would wait on a store
                        osb = opool.tile([HD, 512], F32, tag=f"osb{k}",
                                         bufs=4, name="osb")
                        nc.vector.tensor_mul(
                            osb, pvs[k][0:HD, :], store[f"bc{k}"])
                        h = 2 * t + k
                        nc.sync.dma_start(
                            out=outT_ext[HD * h:HD * (h + 1), ibs], in_=osb)
                return emit

            qkT = (qT0, kT0)
            future = {}
            for t in range(8):
                qT, kT = qkT
                if t < 7:
                    qT_next = qkpool.tile([128, N], BF16, tag="qT", name="qT")
                    kT_next = qkpool.tile([128, N], BF16, tag="kT", name="kT")
                    qk_targets[t + 1] = (qT_next, kT_next)
                for ib in range(2):
                    ibs = slice(ib * 512, (ib + 1) * 512)
                    pvs = {
                        hh: psum.tile([HD + 1, 512], F32, tag="pv", bufs=2,
                                      name=f"pv{hh}")
                        for hh in range(2)
                    }
                    store = {}
                    for g in range(4):
                        c = t * 8 + ib * 4 + g
                        sps = [ps_big(f"sps{hh}") for hh in range(2)]
                        # two heads' score matmuls per j-tile emitted
                        # back-to-back: 64-row PE tiles run concurrently
                        for jh in range(2):
                            jt = 2 * g + jh
                            for hh in range(2):
                                p0 = hh * 64
                                nc.tensor.matmul(
                                    sps[hh][:, jh * 512:(jh + 1) * 512],
                                    kT[p0:p0 + 64, jt * 128:(jt + 1) * 128],
                                    qT[p0:p0 + 64, ibs],
                                    start=True,
                                    stop=True,
                                )
                        es = []
                        for hh in range(2):
                            e = epool.tile([128, 2, 512], BF16, tag=f"E{hh}",
                                           name=f"E{hh}")
                            nc.scalar.activation(
                                out=e.rearrange("p j f -> p (j f)"),
                                in_=sps[hh], func=EXP, scale=SCALE,
                            )
                            es.append(e)
                        # wv dvb1 odd c-tiles ride the scalar ring, issued
                        # in the first chunks where the ACT engine idles
                        # (PE-bound era); frees ~0.5MB from the sync ring
                        if c in (0, 1):
                            for vct in (1, 3) if c == 0 else (5, 7):
                                nc.scalar.dma_start(
                                    out=wv_sb[:, vct, 512:1024],
                                    in_=wv3[:, vct, 512:1024])
                        ems = future.pop(c, [])
                        ems.append(pv_emit(es, g, pvs, t))
                        if g == 3:
                            ems.append(rcp_emit(pvs, store))
                            future.setdefault(c + 1, []).append(
                                bcast_emit(store))
                            future.setdefault(c + 3, []).append(
                                mul_out_emit(pvs, store, t, ibs))
                        pending.append(ems)
                        drain(LAG)
                        for spec in sched.get(c, []):
                            emit_quarter(spec)
                if t < 7:
                    qkT = (qT_next, kT_next)
            drain(0)
            for c in sorted(future):
                for e in future[c]:
                    e()
    nc.compile()
    return nc


_NC_CACHE = {}


def _get_nc():
    if "nc" not in _NC_CACHE:
        _NC_CACHE["nc"] = build_nc()
    return _NC_CACHE["nc"]


def kernel(x: np.ndarray, qkv_w: np.ndarray, _trace: bool = False):
    B = x.shape[0]
    assert x.shape == (B, N, C) and qkv_w.shape == (3 * C, C)
    bf = ml_dtypes.bfloat16
    # pack q,k weights: [pair, p, co, 256]; c = co*128 + p
    wq = qkv_w[0:C].T.reshape(CT, 128, 8, 128).transpose(2, 1, 0, 3)
    wk = qkv_w[C:2 * C].T.reshape(CT, 128, 8, 128).transpose(2, 1, 0, 3)
    wqk = np.ascontiguousarray(
        np.concatenate([wq, wk], axis=3)).astype(bf)
    wv = np.ascontiguousarray(qkv_w[2 * C:3 * C].T).astype(bf)
    in_maps = [
        {"xT": np.ascontiguousarray(x[b].T).astype(bf), "wqk": wqk, "wv": wv}
        for b in range(B)
    ]
    nc = _get_nc()
    res = run_bass_kernel_spmd(
        nc, in_maps, core_ids=list(range(8)), trace=_trace
    )
    out = np.stack([res.results[b]["outT"].T for b in range(B)])
    if _trace:
        return out, res
    return out


# revision 33
# speedup vs baseline: 1.0112x; 1.0112x over previous
"""Multi-head attention (B=8, N=1024, C=1024, H=16) on 8 TRN2 NeuronCores.

Sharding: data-parallel over batch B=8 -> one batch element per core.
Each core computes, for its batch element:
    qkv = x @ qkv_w.T ; q,k,v split ; per-head softmax(q k^T / sqrt(hd)) v

Device-side layout (all matmuls contract over the SBUF partition dim):
  - host passes xT = x[b].T (bf16)     [C, N]   (c on partitions)
  - host passes wqk packed per head-PAIR: [pair, p, co, 256] (q cols 0:128,
    k cols 128:256) so psq/psk partitions are the pair's stacked head dims
  - qT/kT computed transposed          [128, n] (pair head-dims on partitions)
  - v computed in natural layout       [n, dv], augmented with a ones-column
    so the PV matmul also yields the softmax denominator (row 64)
  - scores S^T=[j,i]: per j-tile the TWO heads' matmuls are emitted
    back-to-back as 64-row PE tiles (tile_position (0,0)/(64,0)) so they run
    CONCURRENTLY on the two halves of the PE array (~2x score throughput)
  - exp fused into the PSUM->SBUF copy on the scalar engine (bf16 out,
    128 ACTs of [128,1024]); its ~147us stream runs just under the PE
  - the v-projection and next-pair qk-projections are chopped into
    [128,512]-output 8-matmul "quarters" and dripped into the chunk stream
    instead of running as a ~34us serial prologue (the early chunks have
    PE slack while the exp pipeline fills)
  - exp table preloaded, gpsimd partition_broadcast library prewarmed, and
    PE warmed with dummy matmuls during the input DMA window; input DMAs
    ordered by first use and spread across four engine queues (xT tiles in
    parallel) so the pair-0 projection is never DMA-paced
  - normalization per (pair, i-block), a short DMA-free chain read straight
    from the PV PSUM (no stage copies, no DRAM bounce):
      g3 slot:  DVE reciprocal_approx_fast on the [1,512] denominator row
                (custom DVE op, ~5x the plain reciprocal, no table/library)
      +1 slot:  gpsimd partition_broadcast -> [64,512]
      +2 slot:  DVE multiply (PV psum rows 0:64 x broadcast) -> DMA out
    The same chain is used for every pair including the last, so the tail
    after the final matmul is only ~4-5us of chain latency.
  - host transposes the returned outT back to [n, c]

PSUM (8 banks): shared tag "ps" 3 x [128,1024] (warmup, score chunks,
projection quarters) = 6 banks, tag "pv" 2 x [65,512] = 2 banks.
"""

import sys
from collections import deque

sys.path.insert(0, "/opt/trn_rl_repo")

import ml_dtypes
import numpy as np

import concourse.bacc as bacc
import concourse.mybir as mybir
import concourse.tile as tile
from concourse.bass_utils import run_bass_kernel_spmd

F32 = mybir.dt.float32
BF16 = mybir.dt.bfloat16
EXP = mybir.ActivationFunctionType.Exp

N = 1024  # tokens
C = 1024  # channels
H = 16    # heads
HD = 64   # head dim
CT = 8    # c tiles of 128
SCALE = HD ** -0.5
LAG = 4   # chunks the PV stream lags behind the score stream


def build_nc():
    nc = bacc.Bacc(None, target_bir_lowering=False)
    xT_ext = nc.declare_dram_parameter("xT", [C, N], BF16, isOutput=False)
    # host-packed qk weights: [pair, p, co, 256] (q cols 0:128, k cols 128:256)
    wqk_ext = nc.declare_dram_parameter("wqk", [8, 128, CT, 256], BF16,
                                        isOutput=False)
    wv_ext = nc.declare_dram_parameter("wv", [C, C], BF16, isOutput=False)
    outT_ext = nc.declare_dram_parameter("outT", [C, N], F32, isOutput=True)

    xT3 = xT_ext.rearrange("(co p) n -> p co n", p=128)    # [128, 8, 1024]
    wv3 = wv_ext.rearrange("(co p) d -> p co d", p=128)    # [128, 8, 1024]

    with tile.TileContext(nc) as tc:
        with (
            tc.tile_pool(name="singles", bufs=1) as singles,
            tc.tile_pool(name="psum", bufs=1, space="PSUM") as psum,
            tc.tile_pool(name="qkpool", bufs=2) as qkpool,
            tc.tile_pool(name="epool", bufs=9) as epool,
            tc.tile_pool(name="opool", bufs=2) as opool,
        ):
            # ---- memsets first: wsb feeds the PE warmup, scratch feeds the
            # exp-table preload and the partition_broadcast library prewarm ----
            wsb = singles.tile([128, 512], BF16)
            nc.vector.memset(wsb, 0.0)
            scratch = singles.tile([1, 16], F32)
            nc.vector.memset(scratch, 0.0)

            # ---- input DMAs: only sync (SP) and scalar (Activation) have
            # HARDWARE DMA rings (~100GB/s each); gpsimd's software path is
            # slow, so it carries just two early xT tiles.  scalar's issues
            # all land in the prologue window (a backed-up ring stalls the
            # queue, which later carries the exp stream). ----
            wqk_head = singles.tile([128, 2, 256], BF16)
            nc.sync.dma_start(out=wqk_head, in_=wqk_ext[0, :, 0:2])
            xT_sb = singles.tile([128, CT, N], BF16)
            nc.scalar.dma_start(out=xT_sb[:, 0, :], in_=xT3[:, 0, :])
            nc.gpsimd.dma_start(out=xT_sb[:, 1, :], in_=xT3[:, 1, :])
            nc.scalar.dma_start(out=xT_sb[:, 2, :], in_=xT3[:, 2, :])
            wqk_rest = singles.tile([128, 6, 256], BF16)
            nc.sync.dma_start(out=wqk_rest, in_=wqk_ext[0, :, 2:8])
            nc.gpsimd.dma_start(out=xT_sb[:, 3, :], in_=xT3[:, 3, :])
            for ct in range(4, CT):
                nc.sync.dma_start(out=xT_sb[:, ct, :], in_=xT3[:, ct, :])

            # ---- exp table preload (tiny ACT) + gpsimd partition_broadcast
            # library prewarm, both during the DMA window ----
            scratch_o = singles.tile([1, 16], BF16)
            nc.scalar.activation(out=scratch_o, in_=scratch, func=EXP,
                                 scale=1.0)
            pb_warm = singles.tile([2, 16], F32)
            nc.gpsimd.partition_broadcast(pb_warm, scratch)

            # ---- PE warmup: dummy matmuls during the input-DMA window so
            # the HAM clock gate is already ramping when real work starts ----
            wps = psum.tile([128, 1024], F32, tag="ps", bufs=3, name="warm")
            for _ in range(4):
                nc.tensor.matmul(wps[:, 0:512], wsb[:, 0:128], wsb,
                                 start=True, stop=True,
                                 skip_group_check=True)

            # ---- remaining bulk loads on sync, first-use order: wv dvb0,
            # wqk pair 1, wv dvb1 evens, wqk pairs 2-7 (wv dvb1 odds are
            # issued from the scalar queue inside the first chunk, where the
            # ACT engine is idle anyway -- see the chunk loop) ----
            wv_sb = singles.tile([128, CT, C], BF16)
            for ct in range(CT):
                nc.sync.dma_start(out=wv_sb[:, ct, 0:512],
                                  in_=wv3[:, ct, 0:512])
            wqk_tiles = {}

            def load_wqk(t, q=None):
                w = singles.tile([128, CT, 256], BF16)
                (q or nc.sync).dma_start(out=w, in_=wqk_ext[t])
                wqk_tiles[t] = w

            load_wqk(1)
            # the sync ring runs ~2us behind every wqk deadline; the scalar
            # ring has slack for the next three pairs' weights
            load_wqk(2, q=nc.scalar)
            load_wqk(3, q=nc.scalar)
            load_wqk(4, q=nc.scalar)
            for ct in (0, 2, 4, 6):
                nc.sync.dma_start(out=wv_sb[:, ct, 512:1024],
                                  in_=wv3[:, ct, 512:1024])
            for t in range(5, 8):
                load_wqk(t)

            def wqk0_slice(ct, off):
                if ct < 2:
                    return wqk_head[:, ct, off:off + 128]
                return wqk_rest[:, ct - 2, off:off + 128]

            # v_aug[p, nt, h, 0:64] = v head h rows; v_aug[p, nt, h, 64] = 1.0
            v_aug = singles.tile([128, CT, H, HD + 1], BF16)
            ones16 = singles.tile([128, H], F32)
            nc.vector.memset(ones16, 1.0)
            nc.vector.tensor_copy(
                v_aug[:, :, :, HD],
                ones16[:, None, :].to_broadcast([128, CT, H]),
            )

            def ps_big(name):
                return psum.tile([128, 1024], F32, tag="ps", bufs=3, name=name)

            # ---- pair-0 qk projection (q and k interleaved per c-tile so
            # both ride the incoming xT DMA wave) ----
            psq = ps_big("qk0q")
            psk = ps_big("qk0k")
            for ct in range(CT):
                for ps, off in ((psq, 0), (psk, 128)):
                    for nb in range(2):
                        nc.tensor.matmul(
                            ps[:, nb * 512:(nb + 1) * 512],
                            wqk0_slice(ct, off),
                            xT_sb[:, ct, nb * 512:(nb + 1) * 512],
                            start=(ct == 0),
                            stop=(ct == CT - 1),
                            skip_group_check=True,
                        )
            # qT0 via the (idle) ACT engine, kT0 via the DVE -- concurrent,
            # halving the serial PSUM->SBUF gap between pair-0's last matmul
            # and the first score chunk
            qT0 = qkpool.tile([128, N], BF16, tag="qT", name="qT")
            kT0 = qkpool.tile([128, N], BF16, tag="kT", name="kT")
            nc.scalar.copy(qT0, psq)
            nc.vector.tensor_copy(kT0, psk)

            # ---- projection "quarters": 8-matmul bursts with a [128,512]
            # output, dripped into the chunk stream ----
            qk_targets = {}

            def emit_quarter(spec):
                qps = psum.tile([128, 1024], F32, tag="ps", bufs=3,
                                name="qtr")
                if spec[0] == "qk":
                    _, tn, half, nb = spec
                    dst = qk_targets[tn][half]
                    off = 128 * half
                    w = wqk_tiles[tn]
                    for ct in range(CT):
                        nc.tensor.matmul(
                            qps[:, 0:512],
                            w[:, ct, off:off + 128],
                            xT_sb[:, ct, nb * 512:(nb + 1) * 512],
                            start=(ct == 0),
                            stop=(ct == CT - 1),
                            skip_group_check=True,
                        )
                    nc.vector.tensor_copy(dst[:, nb * 512:(nb + 1) * 512],
                                          qps[:, 0:512])
                else:
                    _, nt, dvb = spec
                    for ct in range(CT):
                        nc.tensor.matmul(
                            qps[:, 0:512],
                            xT_sb[:, ct, nt * 128:(nt + 1) * 128],
                            wv_sb[:, ct, dvb * 512:(dvb + 1) * 512],
                            start=(ct == 0),
                            stop=(ct == CT - 1),
                            skip_group_check=True,
                        )
                    h0 = dvb * 8
                    nc.vector.tensor_copy(
                        v_aug[:, nt, h0:h0 + 8, 0:HD],
                        qps[:, 0:512].rearrange("p (h e) -> p h e", h=8),
                    )

            # quarter schedule by global chunk index c = t*8 + ib*4 + g.
            # t0: v-dvb0 for all 8 token-tiles + pair-1 qk, each as late as
            # its consumer allows (v(nt) feeds the PV of chunk nt//2+LAG;
            # qk1 feeds pair-1 scores at chunk 8) so none outruns the input
            # DMA stream; later t's: pair-(t+1) qk at even slots, v-dvb1
            # dripped during t1-t3.
            sched = {
                0: [("v", 0, 0), ("v", 1, 0)],
                1: [("v", 2, 0), ("qk", 1, 0, 0)],
                2: [("v", 3, 0), ("v", 4, 0)],
                3: [("v", 5, 0), ("qk", 1, 0, 1)],
                4: [("v", 6, 0), ("v", 7, 0)],
                5: [("qk", 1, 1, 0)],
                6: [("qk", 1, 1, 1)],
            }
            vd1 = deque(("v", nt, 1) for nt in range(CT))
            for t in range(1, 7):
                base = t * 8
                for i, (half, nb) in enumerate(
                        ((0, 0), (0, 1), (1, 0), (1, 1))):
                    sched.setdefault(base + 2 * i, []).append(
                        ("qk", t + 1, half, nb))
                # v-dvb1 rides t2-t3 (its wv arrives ~60us; true deadline is
                # t4's PV at ~120us)
                if t in (2, 3):
                    for i in (1, 3, 5, 7):
                        if vd1:
                            sched.setdefault(base + i, []).append(
                                vd1.popleft())
            assert not vd1

            # ---- software-pipelined attention stream ----
            pending = deque()  # entries: list of emitters for one chunk

            def drain(keep):
                while len(pending) > keep:
                    for e in pending.popleft():
                        e()

            def pv_emit(es, g, pvs, t):
                def emit():
                    for hh in range(2):
                        h = 2 * t + hh
                        for jh in range(2):
                            jt = 2 * g + jh
                            nc.tensor.matmul(
                                pvs[hh],
                                v_aug[:, jt, h, :],
                                es[hh][:, jh, :],
                                start=(g == 0 and jh == 0),
                                stop=(g == 3 and jh == 1),
                                skip_group_check=True,
                            )
                return emit

            # Normalization, a short DMA-free chain straight off the PV psum
            # (row 64 is the softmax denominator), spread over 3 chunk-slots
            # so no in-order engine queue ever blocks on an upstream engine:
            #   g3 slot:  DVE reciprocal_approx_fast on the [1,512] row
            #   +1 slot:  gpsimd partition_broadcast -> [64,512]
            #   +2 slot:  DVE multiply (psum x broadcast) + per-head DMA out
            # The multiply is the last reader of the pv psum tiles, freeing
            # them 3 chunk-slots before the slot is needed again (bufs=2).
            def rcp_emit(pvs, store):
                def emit():
                    for k in range(2):
                        # custom-DVE ops misread PSUM: bounce the denominator
                        # row through SBUF with a plain copy first
                        row = opool.tile([1, 512], F32, tag=f"row{k}",
                                         bufs=2, name="row")
                        nc.vector.tensor_copy(row, pvs[k][HD:HD + 1, :])
                        rcp = opool.tile([1, 512], F32, tag=f"rcp{k}",
                                         bufs=2, name="rcp")
                        nc.vector.reciprocal_approx_fast(out=rcp, in_=row)
                        store[f"rcp{k}"] = rcp
                return emit

            def bcast_emit(store):
                def emit():
                    for k in range(2):
                        bc = opool.tile([HD, 512], F32, tag=f"bc{k}",
                                        bufs=2, name="bc")
                        nc.gpsimd.partition_broadcast(bc, store[f"rcp{k}"])
                        store[f"bc{k}"] = bc
                return emit

            def mul_out_emit(pvs, store, t, ibs):
                """The multiply stays on the DVE (gpsimd has no PSUM port),
                but is scheduled one extra chunk-slot after the broadcast so
                its wait on gpsimd is already satisfied when the DVE reaches
                it -- otherwise the wait head-of-line-blocks the PE-critical
                quarter copies (observed as a ~3.5us PE gap + HAM re-throttle
                at every other ib boundary)."""
                def emit():
                    for k in range(2):
                        # bufs=4: early stores queue behind the input tail
                        # on the sync ring; 4 osb slots give the mul ~37us
                        # of slack before it would wait on a store
                        osb = opool.tile([HD, 512], F32, tag=f"osb{k}",
                                         bufs=4, name="osb")
                        nc.vector.tensor_mul(
                            osb, pvs[k][0:HD, :], store[f"bc{k}"])
                        h = 2 * t + k
                        nc.sync.dma_start(
                            out=outT_ext[HD * h:HD * (h + 1), ibs], in_=osb)
                return emit

            qkT = (qT0, kT0)
            future = {}
            for t in range(8):
                qT, kT = qkT
                if t < 7:
                    qT_next = qkpool.tile([128, N], BF16, tag="qT", name="qT")
                    kT_next = qkpool.tile([128, N], BF16, tag="kT", name="kT")
                    qk_targets[t + 1] = (qT_next, kT_next)
                for ib in range(2):
                    ibs = slice(ib * 512, (ib + 1) * 512)
                    pvs = {
                        hh: psum.tile([HD + 1, 512], F32, tag="pv", bufs=2,
                                      name=f"pv{hh}")
                        for hh in range(2)
                    }
                    store = {}
                    for g in range(4):
                        c = t * 8 + ib * 4 + g
                        sps = [ps_big(f"sps{hh}") for hh in range(2)]
                        # two heads' score matmuls per j-tile emitted
                        # back-to-back: 64-row PE tiles run concurrently
                        for jh in range(2):
                            jt = 2 * g + jh
                            for hh in range(2):
                                p0 = hh * 64
                                nc.tensor.matmul(
                                    sps[hh][:, jh * 512:(jh + 1) * 512],
                                    kT[p0:p0 + 64, jt * 128:(jt + 1) * 128],
                                    qT[p0:p0 + 64, ibs],
                                    start=True,
                                    stop=True,
                                )
                        es = []
                        for hh in range(2):
                            e = epool.tile([128, 2, 512], BF16, tag=f"E{hh}",
                                           name=f"E{hh}")
                            nc.scalar.activation(
                                out=e.rearrange("p j f -> p (j f)"),
                                in_=sps[hh], func=EXP, scale=SCALE,
                            )
                            es.append(e)
                        # wv dvb1 odd c-tiles ride the scalar ring, issued
                        # in the first chunks where the ACT engine idles
                        # (PE-bound era); frees ~0.5MB from the sync ring
                        if c in (0, 1):
                            for vct in (1, 3) if c == 0 else (5, 7):
                                nc.scalar.dma_start(
                                    out=wv_sb[:, vct, 512:1024],
                                    in_=wv3[:, vct, 512:1024])
                        ems = future.pop(c, [])
                        ems.append(pv_emit(es, g, pvs, t))
                        if g == 3:
                            ems.append(rcp_emit(pvs, store))
                            future.setdefault(c + 1, []).append(
                                bcast_emit(store))
                            # in the ACT-bound endgame the whole pipeline is
                            # exp-gated and a +3 mul arrives too late for the
                            # next ib's PV (pv psum aliasing); the DVE is
                            # idle there (no quarter copies), so its wait on
                            # gpsimd can't head-of-line-block anything
                            mul_slot = 1 if t >= 6 else 3
                            future.setdefault(c + mul_slot, []).append(
                                mul_out_emit(pvs, store, t, ibs))
                        pending.append(ems)
                        drain(LAG)
                        for spec in sched.get(c, []):
                            emit_quarter(spec)
                if t < 7:
                    qkT = (qT_next, kT_next)
            drain(0)
            for c in sorted(future):
                for e in future[c]:
                    e()
    nc.compile()
    return nc


_NC_CACHE = {}


def _get_nc():
    if "nc" not in _NC_CACHE:
        _NC_CACHE["nc"] = build_nc()
    return _NC_CACHE["nc"]


def kernel(x: np.ndarray, qkv_w: np.ndarray, _trace: bool = False):
    B = x.shape[0]
    assert x.shape == (B, N, C) and qkv_w.shape == (3 * C, C)
    bf = ml_dtypes.bfloat16
    # pack q,k weights: [pair, p, co, 256]; c = co*128 + p
    wq = qkv_w[0:C].T.reshape(CT, 128, 8, 128).transpose(2, 1, 0, 3)
    wk = qkv_w[C:2 * C].T.reshape(CT, 128, 8, 128).transpose(2, 1, 0, 3)
    wqk = np.ascontiguousarray(
        np.concatenate([wq, wk], axis=3)).astype(bf)
    wv = np.ascontiguousarray(qkv_w[2 * C:3 * C].T).astype(bf)
    in_maps = [
        {"xT": np.ascontiguousarray(x[b].T).astype(bf), "wqk": wqk, "wv": wv}
        for b in range(B)
    ]
    nc = _get_nc()
    res = run_bass_kernel_spmd(
        nc, in_maps, core_ids=list(range(8)), trace=_trace
    )
    out = np.stack([res.results[b]["outT"].T for b in range(B)])
    if _trace:
        return out, res
    return out


# revision 36
# speedup vs baseline: 1.0127x; 1.0015x over previous
"""Multi-head attention (B=8, N=1024, C=1024, H=16) on 8 TRN2 NeuronCores.

Sharding: data-parallel over batch B=8 -> one batch element per core.
Each core computes, for its batch element:
    qkv = x @ qkv_w.T ; q,k,v split ; per-head softmax(q k^T / sqrt(hd)) v

Device-side layout (all matmuls contract over the SBUF partition dim):
  - host passes xT = x[b].T (bf16)     [C, N]   (c on partitions)
  - host passes wqk packed per head-PAIR: [pair, p, co, 256] (q cols 0:128,
    k cols 128:256) so psq/psk partitions are the pair's stacked head dims
  - qT/kT computed transposed          [128, n] (pair head-dims on partitions)
  - v computed in natural layout       [n, dv], augmented with a ones-column
    so the PV matmul also yields the softmax denominator (row 64)
  - scores S^T=[j,i]: per j-tile the TWO heads' matmuls are emitted
    back-to-back as 64-row PE tiles (tile_position (0,0)/(64,0)) so they run
    CONCURRENTLY on the two halves of the PE array (~2x score throughput)
  - exp fused into the PSUM->SBUF copy on the scalar engine (bf16 out,
    128 ACTs of [128,1024]); its ~147us stream runs just under the PE
  - the v-projection and next-pair qk-projections are chopped into
    [128,512]-output 8-matmul "quarters" and dripped into the chunk stream
    instead of running as a ~34us serial prologue (the early chunks have
    PE slack while the exp pipeline fills)
  - exp table preloaded, gpsimd partition_broadcast library prewarmed, and
    PE warmed with dummy matmuls during the input DMA window; input DMAs
    ordered by first use and spread across four engine queues (xT tiles in
    parallel) so the pair-0 projection is never DMA-paced
  - normalization per (pair, i-block), a short DMA-free chain read straight
    from the PV PSUM (no stage copies, no DRAM bounce):
      g3 slot:  DVE reciprocal_approx_fast on the [1,512] denominator row
                (custom DVE op, ~5x the plain reciprocal, no table/library)
      +1 slot:  gpsimd partition_broadcast -> [64,512]
      +2 slot:  DVE multiply (PV psum rows 0:64 x broadcast) -> DMA out
    The same chain is used for every pair including the last, so the tail
    after the final matmul is only ~4-5us of chain latency.
  - host transposes the returned outT back to [n, c]

PSUM (8 banks): shared tag "ps" 3 x [128,1024] (warmup, score chunks,
projection quarters) = 6 banks, tag "pv" 2 x [65,512] = 2 banks.
"""

import sys
from collections import deque

sys.path.insert(0, "/opt/trn_rl_repo")

import ml_dtypes
import numpy as np

import concourse.bacc as bacc
import concourse.mybir as mybir
import concourse.tile as tile
from concourse.bass_utils import run_bass_kernel_spmd

F32 = mybir.dt.float32
BF16 = mybir.dt.bfloat16
EXP = mybir.ActivationFunctionType.Exp

N = 1024  # tokens
C = 1024  # channels
H = 16    # heads
HD = 64   # head dim
CT = 8    # c tiles of 128
SCALE = HD ** -0.5
LAG = 4   # chunks the PV stream lags behind the score stream


def build_nc():
    nc = bacc.Bacc(None, target_bir_lowering=False)
    xT_ext = nc.declare_dram_parameter("xT", [C, N], BF16, isOutput=False)
    # host-packed qk weights: [pair, p, co, 256] (q cols 0:128, k cols 128:256)
    wqk_ext = nc.declare_dram_parameter("wqk", [8, 128, CT, 256], BF16,
                                        isOutput=False)
    wv_ext = nc.declare_dram_parameter("wv", [C, C], BF16, isOutput=False)
    outT_ext = nc.declare_dram_parameter("outT", [C, N], F32, isOutput=True)

    xT3 = xT_ext.rearrange("(co p) n -> p co n", p=128)    # [128, 8, 1024]
    wv3 = wv_ext.rearrange("(co p) d -> p co d", p=128)    # [128, 8, 1024]

    with tile.TileContext(nc) as tc:
        with (
            tc.tile_pool(name="singles", bufs=1) as singles,
            tc.tile_pool(name="psum", bufs=1, space="PSUM") as psum,
            tc.tile_pool(name="qkpool", bufs=2) as qkpool,
            tc.tile_pool(name="epool", bufs=9) as epool,
            tc.tile_pool(name="opool", bufs=2) as opool,
        ):
            # ---- memsets first: wsb feeds the PE warmup, scratch feeds the
            # exp-table preload and the partition_broadcast library prewarm ----
            wsb = singles.tile([128, 512], BF16)
            nc.vector.memset(wsb, 0.0)
            scratch = singles.tile([1, 16], F32)
            nc.vector.memset(scratch, 0.0)

            # ---- input DMAs: only sync (SP) and scalar (Activation) have
            # HARDWARE DMA rings (~100GB/s each); gpsimd's software path is
            # slow, so it carries just two early xT tiles.  scalar's issues
            # all land in the prologue window (a backed-up ring stalls the
            # queue, which later carries the exp stream). ----
            wqk_head = singles.tile([128, 2, 256], BF16)
            nc.sync.dma_start(out=wqk_head, in_=wqk_ext[0, :, 0:2])
            xT_sb = singles.tile([128, CT, N], BF16)
            nc.scalar.dma_start(out=xT_sb[:, 0, :], in_=xT3[:, 0, :])
            nc.gpsimd.dma_start(out=xT_sb[:, 1, :], in_=xT3[:, 1, :])
            nc.scalar.dma_start(out=xT_sb[:, 2, :], in_=xT3[:, 2, :])
            wqk_rest = singles.tile([128, 6, 256], BF16)
            nc.sync.dma_start(out=wqk_rest, in_=wqk_ext[0, :, 2:8])
            nc.gpsimd.dma_start(out=xT_sb[:, 3, :], in_=xT3[:, 3, :])
            for ct in range(4, CT):
                nc.sync.dma_start(out=xT_sb[:, ct, :], in_=xT3[:, ct, :])

            # ---- exp table preload (tiny ACT) + gpsimd partition_broadcast
            # library prewarm, both during the DMA window ----
            scratch_o = singles.tile([1, 16], BF16)
            nc.scalar.activation(out=scratch_o, in_=scratch, func=EXP,
                                 scale=1.0)
            pb_warm = singles.tile([2, 16], F32)
            nc.gpsimd.partition_broadcast(pb_warm, scratch)

            # ---- PE warmup: dummy matmuls during the input-DMA window so
            # the HAM clock gate is already ramping when real work starts ----
            wps = psum.tile([128, 1024], F32, tag="ps", bufs=3, name="warm")
            for _ in range(4):
                nc.tensor.matmul(wps[:, 0:512], wsb[:, 0:128], wsb,
                                 start=True, stop=True,
                                 skip_group_check=True)

            # ---- remaining bulk loads on sync, first-use order: wv dvb0,
            # wqk pair 1, wv dvb1 evens, wqk pairs 2-7 (wv dvb1 odds are
            # issued from the scalar queue inside the first chunk, where the
            # ACT engine is idle anyway -- see the chunk loop) ----
            wv_sb = singles.tile([128, CT, C], BF16)
            for ct in range(CT):
                nc.sync.dma_start(out=wv_sb[:, ct, 0:512],
                                  in_=wv3[:, ct, 0:512])
            wqk_tiles = {}

            def load_wqk(t, q=None):
                w = singles.tile([128, CT, 256], BF16)
                (q or nc.sync).dma_start(out=w, in_=wqk_ext[t])
                wqk_tiles[t] = w

            load_wqk(1)
            # the sync ring runs ~2us behind every wqk deadline; the scalar
            # ring (done by ~50us) has slack for four more pairs' weights
            load_wqk(2, q=nc.scalar)
            load_wqk(3, q=nc.scalar)
            load_wqk(4, q=nc.scalar)
            load_wqk(5, q=nc.scalar)
            for ct in (0, 2, 4, 6):
                nc.sync.dma_start(out=wv_sb[:, ct, 512:1024],
                                  in_=wv3[:, ct, 512:1024])
            for t in range(6, 8):
                load_wqk(t)

            def wqk0_slice(ct, off):
                if ct < 2:
                    return wqk_head[:, ct, off:off + 128]
                return wqk_rest[:, ct - 2, off:off + 128]

            # v_aug[p, nt, h, 0:64] = v head h rows; v_aug[p, nt, h, 64] = 1.0
            v_aug = singles.tile([128, CT, H, HD + 1], BF16)
            ones16 = singles.tile([128, H], F32)
            nc.vector.memset(ones16, 1.0)
            nc.vector.tensor_copy(
                v_aug[:, :, :, HD],
                ones16[:, None, :].to_broadcast([128, CT, H]),
            )

            def ps_big(name):
                return psum.tile([128, 1024], F32, tag="ps", bufs=3, name=name)

            # ---- pair-0 qk projection (q and k interleaved per c-tile so
            # both ride the incoming xT DMA wave) ----
            psq = ps_big("qk0q")
            psk = ps_big("qk0k")
            for ct in range(CT):
                for ps, off in ((psq, 0), (psk, 128)):
                    for nb in range(2):
                        nc.tensor.matmul(
                            ps[:, nb * 512:(nb + 1) * 512],
                            wqk0_slice(ct, off),
                            xT_sb[:, ct, nb * 512:(nb + 1) * 512],
                            start=(ct == 0),
                            stop=(ct == CT - 1),
                            skip_group_check=True,
                        )
            # qT0 via the (idle) ACT engine, kT0 via the DVE -- concurrent,
            # halving the serial PSUM->SBUF gap between pair-0's last matmul
            # and the first score chunk
            qT0 = qkpool.tile([128, N], BF16, tag="qT", name="qT")
            kT0 = qkpool.tile([128, N], BF16, tag="kT", name="kT")
            nc.scalar.copy(qT0, psq)
            nc.vector.tensor_copy(kT0, psk)

            # ---- projection "quarters": 8-matmul bursts with a [128,512]
            # output, dripped into the chunk stream ----
            qk_targets = {}

            def emit_quarter(spec):
                qps = psum.tile([128, 1024], F32, tag="ps", bufs=3,
                                name="qtr")
                if spec[0] == "qk":
                    _, tn, half, nb = spec
                    dst = qk_targets[tn][half]
                    off = 128 * half
                    w = wqk_tiles[tn]
                    for ct in range(CT):
                        nc.tensor.matmul(
                            qps[:, 0:512],
                            w[:, ct, off:off + 128],
                            xT_sb[:, ct, nb * 512:(nb + 1) * 512],
                            start=(ct == 0),
                            stop=(ct == CT - 1),
                            skip_group_check=True,
                        )
                    nc.vector.tensor_copy(dst[:, nb * 512:(nb + 1) * 512],
                                          qps[:, 0:512])
                else:
                    _, nt, dvb = spec
                    for ct in range(CT):
                        nc.tensor.matmul(
                            qps[:, 0:512],
                            xT_sb[:, ct, nt * 128:(nt + 1) * 128],
                            wv_sb[:, ct, dvb * 512:(dvb + 1) * 512],
                            start=(ct == 0),
                            stop=(ct == CT - 1),
                            skip_group_check=True,
                        )
                    h0 = dvb * 8
                    nc.vector.tensor_copy(
                        v_aug[:, nt, h0:h0 + 8, 0:HD],
                        qps[:, 0:512].rearrange("p (h e) -> p h e", h=8),
                    )

            # quarter schedule by global chunk index c = t*8 + ib*4 + g.
            # t0: v-dvb0 for all 8 token-tiles + pair-1 qk, each as late as
            # its consumer allows (v(nt) feeds the PV of chunk nt//2+LAG;
            # qk1 feeds pair-1 scores at chunk 8) so none outruns the input
            # DMA stream; later t's: pair-(t+1) qk at even slots, v-dvb1
            # dripped during t1-t3.
            sched = {
                0: [("v", 0, 0), ("v", 1, 0)],
                1: [("v", 2, 0), ("qk", 1, 0, 0)],
                2: [("v", 3, 0), ("v", 4, 0)],
                3: [("v", 5, 0), ("qk", 1, 0, 1)],
                4: [("v", 6, 0), ("v", 7, 0)],
                5: [("qk", 1, 1, 0)],
                6: [("qk", 1, 1, 1)],
            }
            vd1 = deque(("v", nt, 1) for nt in range(CT))
            for t in range(1, 7):
                base = t * 8
                for i, (half, nb) in enumerate(
                        ((0, 0), (0, 1), (1, 0), (1, 1))):
                    sched.setdefault(base + 2 * i, []).append(
                        ("qk", t + 1, half, nb))
                # v-dvb1 rides t2-t3 (its wv arrives ~60us; true deadline is
                # t4's PV at ~120us)
                if t in (2, 3):
                    for i in (1, 3, 5, 7):
                        if vd1:
                            sched.setdefault(base + i, []).append(
                                vd1.popleft())
            assert not vd1

            # ---- software-pipelined attention stream ----
            pending = deque()  # entries: list of emitters for one chunk

            def drain(keep):
                while len(pending) > keep:
                    for e in pending.popleft():
                        e()

            def pv_emit(es, g, pvs, t):
                def emit():
                    for hh in range(2):
                        h = 2 * t + hh
                        for jh in range(2):
                            jt = 2 * g + jh
                            nc.tensor.matmul(
                                pvs[hh],
                                v_aug[:, jt, h, :],
                                es[hh][:, jh, :],
                                start=(g == 0 and jh == 0),
                                stop=(g == 3 and jh == 1),
                                skip_group_check=True,
                            )
                return emit

            # Normalization, a short DMA-free chain straight off the PV psum
            # (row 64 is the softmax denominator), spread over 3 chunk-slots
            # so no in-order engine queue ever blocks on an upstream engine:
            #   g3 slot:  DVE reciprocal_approx_fast on the [1,512] row
            #   +1 slot:  gpsimd partition_broadcast -> [64,512]
            #   +2 slot:  DVE multiply (psum x broadcast) + per-head DMA out
            # The multiply is the last reader of the pv psum tiles, freeing
            # them 3 chunk-slots before the slot is needed again (bufs=2).
            def rcp_emit(pvs, store):
                def emit():
                    for k in range(2):
                        # custom-DVE ops misread PSUM: bounce the denominator
                        # row through SBUF with a plain copy first
                        row = opool.tile([1, 512], F32, tag=f"row{k}",
                                         bufs=2, name="row")
                        nc.vector.tensor_copy(row, pvs[k][HD:HD + 1, :])
                        rcp = opool.tile([1, 512], F32, tag=f"rcp{k}",
                                         bufs=2, name="rcp")
                        nc.vector.reciprocal_approx_fast(out=rcp, in_=row)
                        store[f"rcp{k}"] = rcp
                return emit

            def bcast_emit(store):
                def emit():
                    for k in range(2):
                        bc = opool.tile([HD, 512], F32, tag=f"bc{k}",
                                        bufs=2, name="bc")
                        nc.gpsimd.partition_broadcast(bc, store[f"rcp{k}"])
                        store[f"bc{k}"] = bc
                return emit

            def mul_out_emit(pvs, store, t, ibs):
                """The multiply stays on the DVE (gpsimd has no PSUM port),
                but is scheduled one extra chunk-slot after the broadcast so
                its wait on gpsimd is already satisfied when the DVE reaches
                it -- otherwise the wait head-of-line-blocks the PE-critical
                quarter copies (observed as a ~3.5us PE gap + HAM re-throttle
                at every other ib boundary)."""
                def emit():
                    for k in range(2):
                        # bufs=4: early stores queue behind the input tail
                        # on the sync ring; 4 osb slots give the mul ~37us
                        # of slack before it would wait on a store
                        osb = opool.tile([HD, 512], F32, tag=f"osb{k}",
                                         bufs=4, name="osb")
                        nc.vector.tensor_mul(
                            osb, pvs[k][0:HD, :], store[f"bc{k}"])
                        h = 2 * t + k
                        # the final ib's stores go out on the scalar ring:
                        # the exp stream is over, its queue and ring are
                        # idle, while sync still drains earlier stores
                        q = nc.scalar if (t == 7 and ibs.start == 512) \
                            else nc.sync
                        q.dma_start(
                            out=outT_ext[HD * h:HD * (h + 1), ibs], in_=osb)
                return emit

            qkT = (qT0, kT0)
            future = {}
            for t in range(8):
                qT, kT = qkT
                if t < 7:
                    qT_next = qkpool.tile([128, N], BF16, tag="qT", name="qT")
                    kT_next = qkpool.tile([128, N], BF16, tag="kT", name="kT")
                    qk_targets[t + 1] = (qT_next, kT_next)
                for ib in range(2):
                    ibs = slice(ib * 512, (ib + 1) * 512)
                    pvs = {
                        hh: psum.tile([HD + 1, 512], F32, tag="pv", bufs=2,
                                      name=f"pv{hh}")
                        for hh in range(2)
                    }
                    store = {}
                    for g in range(4):
                        c = t * 8 + ib * 4 + g
                        sps = [ps_big(f"sps{hh}") for hh in range(2)]
                        # two heads' score matmuls per j-tile emitted
                        # back-to-back: 64-row PE tiles run concurrently
                        for jh in range(2):
                            jt = 2 * g + jh
                            for hh in range(2):
                                p0 = hh * 64
                                nc.tensor.matmul(
                                    sps[hh][:, jh * 512:(jh + 1) * 512],
                                    kT[p0:p0 + 64, jt * 128:(jt + 1) * 128],
                                    qT[p0:p0 + 64, ibs],
                                    start=True,
                                    stop=True,
                                )
                        es = []
                        for hh in range(2):
                            e = epool.tile([128, 2, 512], BF16, tag=f"E{hh}",
                                           name=f"E{hh}")
                            nc.scalar.activation(
                                out=e.rearrange("p j f -> p (j f)"),
                                in_=sps[hh], func=EXP, scale=SCALE,
                            )
                            es.append(e)
                        # wv dvb1 odd c-tiles ride the scalar ring, issued
                        # in early chunks where the ACT engine idles
                        # (PE-bound era); frees ~0.5MB from the sync ring
                        if c in (4, 5):
                            for vct in (1, 3) if c == 4 else (5, 7):
                                nc.scalar.dma_start(
                                    out=wv_sb[:, vct, 512:1024],
                                    in_=wv3[:, vct, 512:1024])
                        ems = future.pop(c, [])
                        ems.append(pv_emit(es, g, pvs, t))
                        if g == 3:
                            ems.append(rcp_emit(pvs, store))
                            future.setdefault(c + 1, []).append(
                                bcast_emit(store))
                            # in the ACT-bound endgame the whole pipeline is
                            # exp-gated and a +3 mul arrives too late for the
                            # next ib's PV (pv psum aliasing); the DVE is
                            # idle there (no quarter copies), so its wait on
                            # gpsimd can't head-of-line-block anything
                            mul_slot = 1 if t >= 6 else 3
                            future.setdefault(c + mul_slot, []).append(
                                mul_out_emit(pvs, store, t, ibs))
                        pending.append(ems)
                        drain(LAG)
                        for spec in sched.get(c, []):
                            emit_quarter(spec)
                if t < 7:
                    qkT = (qT_next, kT_next)
            drain(0)
            for c in sorted(future):
                for e in future[c]:
                    e()
    nc.compile()
    return nc


_NC_CACHE = {}


def _get_nc():
    if "nc" not in _NC_CACHE:
        _NC_CACHE["nc"] = build_nc()
    return _NC_CACHE["nc"]


def kernel(x: np.ndarray, qkv_w: np.ndarray, _trace: bool = False):
    B = x.shape[0]
    assert x.shape == (B, N, C) and qkv_w.shape == (3 * C, C)
    bf = ml_dtypes.bfloat16
    # pack q,k weights: [pair, p, co, 256]; c = co*128 + p
    wq = qkv_w[0:C].T.reshape(CT, 128, 8, 128).transpose(2, 1, 0, 3)
    wk = qkv_w[C:2 * C].T.reshape(CT, 128, 8, 128).transpose(2, 1, 0, 3)
    wqk = np.ascontiguousarray(
        np.concatenate([wq, wk], axis=3)).astype(bf)
    wv = np.ascontiguousarray(qkv_w[2 * C:3 * C].T).astype(bf)
    in_maps = [
        {"xT": np.ascontiguousarray(x[b].T).astype(bf), "wqk": wqk, "wv": wv}
        for b in range(B)
    ]
    nc = _get_nc()
    res = run_bass_kernel_spmd(
        nc, in_maps, core_ids=list(range(8)), trace=_trace
    )
    out = np.stack([res.results[b]["outT"].T for b in range(B)])
    if _trace:
        return out, res
    return out
